# revision 1
# baseline (speedup 1.0000x reference)
"""Tensor-parallel transformer block (attention + MLP with RMSNorm) on 8 TRN2 NeuronCores.

Strategy (TP over 8 cores, everything in transposed [feature, token] layout):
  - c_attn column-sharded by heads (2 heads/core), c_proj row-sharded.
  - attention computed with transposed scores [key, query] so softmax sums
    run on the PE (ones-matmul) and the probs feed A@V with no transposes.
  - AllReduce #1 carries attn_partial + x/8 == x2 in bf16 (4 T-chunks,
    pipelined against attention of later chunks); score matmuls are emitted
    with 2-block lookahead so the PE never stalls on the DVE/ACT softmax.
  - MLP fc1/fc2 column-sharded (704/core, bf16), mlp c_proj row-sharded.
  - ReduceScatter #2 carries mlp_partial + x2/8 == final output, C-sharded;
    host transposes/concats the shards.
Matmuls run in fp32r (attention path) and bf16 (MLP path).
"""
import numpy as np
import ml_dtypes

import concourse.bass as bass
import concourse.tile as tile
from concourse import bacc, mybir
from concourse.bass_utils import run_bass_kernel_spmd
from concourse.masks import make_identity

f32 = mybir.dt.float32
f32r = mybir.dt.float32r
bf16 = mybir.dt.bfloat16
FA = mybir.ActivationFunctionType
ALU = mybir.AluOpType

NCORES = 8
T = 2048
C = 2048
NH = 16
HD = 128
HL = NH // NCORES          # 2 local heads
CL = HL * HD               # 256 local attn dims
DFF = 5632
DL = DFF // NCORES         # 704 local ffn dims
EPS = 1e-5
ISQ = float(1.0 / np.sqrt(HD))
NEG = -30000.0
KC = C // 128              # 16 contraction chunks
NT4 = T // 512             # 4 T-chunks of 512
FCO = [128] * 5 + [64]     # DL partition chunks (704 = 5*128 + 64)
RG = [list(range(NCORES))]


def build(reps=1):
    nc = bacc.Bacc("TRN2", target_bir_lowering=False, debug=False,
                   enable_asserts=False, num_devices=NCORES)

    xt = nc.dram_tensor("xt", [C, T], f32, kind="ExternalInput").ap()
    rs1 = nc.dram_tensor("rs1", [1, T], f32, kind="ExternalInput").ap()
    wqkv = nc.dram_tensor("wqkv", [C, 3 * CL], f32, kind="ExternalInput").ap()
    alibiT = nc.dram_tensor("alibiT", [HL, T, T], bf16, kind="ExternalInput").ap()
    wproj = nc.dram_tensor("wproj", [CL, C], f32, kind="ExternalInput").ap()
    wfc1 = nc.dram_tensor("wfc1", [C, DL], bf16, kind="ExternalInput").ap()
    wfc2 = nc.dram_tensor("wfc2", [C, DL], bf16, kind="ExternalInput").ap()
    wmp = nc.dram_tensor("wmp", [DL, C], bf16, kind="ExternalInput").ap()
    out = nc.dram_tensor("out", [C // NCORES, T], f32, kind="ExternalOutput").ap()


    with tile.TileContext(nc) as tc:
        with tc.tile_pool(name="consts", bufs=1) as consts, \
             tc.tile_pool(name="smalls", bufs=2) as smalls:

            # ---- constants
            ident = consts.tile([128, 128], f32, tag="ident")
            make_identity(nc, ident)
            ones_st = consts.tile([128, 128], f32, tag="ones_st")
            nc.vector.memset(ones_st[:], 1.0)
            ones_col = consts.tile([128, 1], f32r, tag="ones_col")
            nc.scalar.copy(ones_col[:], ones_st[:, :1])
            ones_row = consts.tile([1, 128], f32r, tag="ones_row")
            nc.scalar.copy(ones_row[:], ones_st[:1, :])
            eps_t = consts.tile([1, 1], f32, tag="eps_t")
            nc.vector.memset(eps_t[:], EPS)
            rs1_r = consts.tile([1, T], f32r, tag="rs1_r")
            nc.sync.dma_start(rs1_r[:], rs1[:].bitcast(f32r))

            for rep in range(reps):
              arin = [nc.dram_tensor(f"arin{i}_{rep}", [C, 512], bf16).ap()
                      for i in range(NT4)]
              arout = [nc.dram_tensor(f"arout{i}_{rep}", [C, 512], bf16,
                                      addr_space="Shared").ap()
                       for i in range(NT4)]
              rsin = [nc.dram_tensor(f"rsin{i}_{rep}", [C, 512], f32).ap()
                      for i in range(NT4)]
              rsout = [nc.dram_tensor(f"rsout{i}_{rep}", [C // NCORES, 512],
                                      f32).ap()
                       for i in range(NT4)]
              with tc.tile_pool(name="attn_persist", bufs=1) as apst:
                  # q/k per head (f32r, [HD, T]) — live through attention
                  qk = [apst.tile([128, T], f32r, name=f"qk{i}", tag=f"qk{i}")
                        for i in range(4)]
                  # v per head in normal layout [T(16 chunks of 128), HD]
                  v_sb = [apst.tile([128, KC, 128], f32r, name=f"v{h}", tag=f"v{h}")
                          for h in range(HL)]
                  # attention output transposed, per head [HD, T]
                  yT = [apst.tile([128, T], f32r, name=f"yT{h}", tag=f"yT{h}")
                        for h in range(HL)]

                  # ========== phase A+B: rsinv broadcast, qkv projection ==========
                  with tc.tile_pool(name="rs1bp", bufs=1) as rs1bp:
                      with tc.tile_pool(name="bc_ps", bufs=2, space="PSUM") as bc_ps:
                          rs1b = rs1bp.tile([128, T], f32, tag="rs1b")
                          for t4 in range(NT4):
                              sl = slice(t4 * 512, (t4 + 1) * 512)
                              pb = bc_ps.tile([128, 512], f32, name="pb", tag="pb")
                              nc.tensor.matmul(pb[:], ones_row[:], rs1_r[:, sl],
                                               start=True, stop=True)
                              nc.scalar.copy(rs1b[:, sl], pb[:])

                      with tc.tile_pool(name="wq", bufs=1) as wq, \
                           tc.tile_pool(name="xt_st", bufs=3) as xt_st, \
                           tc.tile_pool(name="xh1", bufs=3) as xh1p, \
                           tc.tile_pool(name="vTp", bufs=1) as vTp, \
                           tc.tile_pool(name="qkv_ps", bufs=1, space="PSUM") as qkv_ps:
                          wq_sb = []
                          for kc in range(KC):
                              w = wq.tile([128, 3 * CL], f32r, name=f"wqr{kc}",
                                          tag=f"wqr{kc}")
                              nc.sync.dma_start(
                                  w[:], wqkv[kc * 128:(kc + 1) * 128, :].bitcast(f32r))
                              wq_sb.append(w)

                          vT = [vTp.tile([128, T], f32r, name=f"vT{h}", tag=f"vT{h}")
                                for h in range(HL)]
                          # destination map for the 6 output chunks of qkv^T
                          dests = [qk[0], qk[1], qk[2], qk[3], vT[0], vT[1]]

                          for t4 in range(NT4):
                              sl = slice(t4 * 512, (t4 + 1) * 512)
                              pss = [qkv_ps.tile([128, 512], f32, name=f"qkvps{oc}",
                                                 tag="qkvps", bufs=6)
                                     for oc in range(6)]
                              for kc in range(KC):
                                  xst = xt_st.tile([128, 512], f32, name="xst", tag="xst")
                                  nc.sync.dma_start(
                                      xst[:], xt[kc * 128:(kc + 1) * 128, sl])
                                  xh = xh1p.tile([128, 512], f32r, name="xh", tag="xh")
                                  nc.vector.tensor_mul(xh[:], xst[:], rs1b[:, sl])
                                  for oc in range(6):
                                      nc.tensor.matmul(
                                          pss[oc][:],
                                          wq_sb[kc][:, oc * 128:(oc + 1) * 128],
                                          xh[:], start=(kc == 0), stop=(kc == KC - 1),
                                          skip_group_check=True)
                              for oc in range(6):
                                  nc.scalar.copy(dests[oc][:, sl], pss[oc][:])

                          # transpose v^T -> v (normal layout)
                          for h in range(HL):
                              for tq in range(KC):
                                  pt = qkv_ps.tile([128, 128], f32, name="vtps",
                                                   tag="vtps", bufs=2)
                                  nc.tensor.transpose(
                                      pt[:],
                                      vT[h][:, tq * 128:(tq + 1) * 128].bitcast(f32),
                                      ident[:])
                                  nc.scalar.copy(v_sb[h][:, tq, :], pt[:])

                  # ======== phase C+D: attention + proj + AllReduce ========
                  with tc.tile_pool(name="wp", bufs=1) as wp, \
                       tc.tile_pool(name="al", bufs=4) as alp, \
                       tc.tile_pool(name="ssb", bufs=3) as ssb, \
                       tc.tile_pool(name="prb", bufs=4) as prb, \
                       tc.tile_pool(name="rbp", bufs=2) as rbp, \
                       tc.tile_pool(name="xtd", bufs=16) as xtd, \
                       tc.tile_pool(name="obp", bufs=3) as obp, \
                       tc.tile_pool(name="at_ps", bufs=3, space="PSUM") as at_ps, \
                       tc.tile_pool(name="ao_ps", bufs=2, space="PSUM") as ao_ps, \
                       tc.tile_pool(name="sum_ps", bufs=1, space="PSUM") as sum_ps, \
                       tc.tile_pool(name="pj_ps", bufs=2, space="PSUM") as pj_ps:

                      wp_sb = []
                      for lc in range(HL):
                          w = wp.tile([128, C], f32r, name=f"wpr{lc}", tag=f"wpr{lc}")
                          nc.sync.dma_start(
                              w[:], wproj[lc * 128:(lc + 1) * 128, :].bitcast(f32r))
                          wp_sb.append(w)

                      for qg in range(NT4):
                          qsl = slice(qg * 512, (qg + 1) * 512)
                          ntk = (qg + 1) * 4
                          for h in range(HL):
                              po = ao_ps.tile([128, 512], f32, name="po", tag="po")
                              psum = sum_ps.tile([1, 512], f32, name="psum", tag="psum")
                              prs = {}

                              def _scores(tkc, h=h, qsl=qsl, qg=qg, prs=prs):
                                  tsl = slice(tkc * 128, (tkc + 1) * 128)
                                  ps_s = at_ps.tile([128, 512], f32, name="ps_s",
                                                    tag="ps_s")
                                  nc.tensor.matmul(ps_s[:], qk[2 + h][:, tsl],
                                                   qk[h][:, qsl], start=True, stop=True,
                                                   skip_group_check=True)
                                  al = alp.tile([128, 512], bf16, name="al", tag="al")
                                  nc.sync.dma_start(al[:], alibiT[h, tsl, qsl])
                                  s_sb = ssb.tile([128, 512], f32, name="s_sb",
                                                  tag="s_sb")
                                  nc.vector.scalar_tensor_tensor(
                                      s_sb[:], ps_s[:], ISQ, al[:],
                                      op0=ALU.mult, op1=ALU.add)
                                  pr = prb.tile([128, 512], f32r, name="pr", tag="pr")
                                  nc.scalar.activation(pr[:], s_sb[:], FA.Exp)
                                  prs[tkc] = pr

                              def _av(tkc, h=h, ntk=ntk, po=po, psum=psum, prs=prs):
                                  pr = prs.pop(tkc)
                                  nc.tensor.matmul(psum[:], ones_col[:], pr[:],
                                                   start=(tkc == 0),
                                                   stop=(tkc == ntk - 1),
                                                   skip_group_check=True)
                                  nc.tensor.matmul(po[:], v_sb[h][:, tkc, :], pr[:],
                                                   start=(tkc == 0),
                                                   stop=(tkc == ntk - 1),
                                                   skip_group_check=True)

                              LOOK = 2
                              for tkc in range(ntk):
                                  _scores(tkc)
                                  if tkc >= LOOK:
                                      _av(tkc - LOOK)
                              for tkc in range(max(0, ntk - LOOK), ntk):
                                  _av(tkc)
                              rc = smalls.tile([1, 512], f32, name="rc", tag="rc")
                              nc.vector.reciprocal(rc[:], psum[:])
                              rcr = smalls.tile([1, 512], f32r, name="rcr", tag="rcr")
                              nc.scalar.copy(rcr[:], rc[:])
                              pbc = at_ps.tile([128, 512], f32, name="pbc", tag="ps_s")
                              nc.tensor.matmul(pbc[:], ones_row[:], rcr[:],
                                               start=True, stop=True,
                                               skip_group_check=True)
                              rb = rbp.tile([128, 512], f32, name="rb", tag="rb")
                              nc.scalar.copy(rb[:], pbc[:])
                              nc.vector.tensor_mul(yT[h][:, qsl], po[:], rb[:])

                          # ---- attn out projection for this T-chunk + residual + AR
                          xts = []
                          for cc in range(KC):
                              xst = xtd.tile([128, 512], f32, name="xtd", tag="xtd")
                              nc.sync.dma_start(xst[:],
                                                xt[cc * 128:(cc + 1) * 128, qsl])
                              xts.append(xst)
                          for cc in range(KC):
                              pj = pj_ps.tile([128, 512], f32, name="pj", tag="pj")
                              for lc in range(HL):
                                  nc.tensor.matmul(
                                      pj[:], wp_sb[lc][:, cc * 128:(cc + 1) * 128],
                                      yT[lc][:, qsl],
                                      start=(lc == 0), stop=(lc == HL - 1))
                              ob = obp.tile([128, 512], bf16, name="ob", tag="ob")
                              nc.vector.scalar_tensor_tensor(
                                  ob[:], xts[cc][:], 1.0 / NCORES, pj[:],
                                  op0=ALU.mult, op1=ALU.add)
                              nc.sync.dma_start(arin[qg][cc * 128:(cc + 1) * 128, :],
                                                ob[:])
                          nc.gpsimd.collective_compute(
                              "AllReduce", ALU.add, ins=[arin[qg][:]],
                              outs=[arout[qg][:]], replica_groups=RG)

              # ================= phase E: MLP + ReduceScatter =================
              with tc.tile_pool(name="wf1", bufs=1) as wf1p, \
                   tc.tile_pool(name="wf2", bufs=1) as wf2p, \
                   tc.tile_pool(name="wm", bufs=1) as wmpl, \
                   tc.tile_pool(name="x2p", bufs=17) as x2p, \
                   tc.tile_pool(name="xh2", bufs=17) as xh2p, \
                   tc.tile_pool(name="xsq", bufs=3) as xsqp, \
                   tc.tile_pool(name="ap2", bufs=2) as ap2, \
                   tc.tile_pool(name="gp", bufs=7) as gp, \
                   tc.tile_pool(name="ob2", bufs=3) as ob2p, \
                   tc.tile_pool(name="rb2p", bufs=2) as rb2p, \
                   tc.tile_pool(name="fc_ps", bufs=4, space="PSUM") as fc_ps, \
                   tc.tile_pool(name="pm_ps", bufs=2, space="PSUM") as pm_ps, \
                   tc.tile_pool(name="st_ps", bufs=1, space="PSUM") as st_ps:

                  wf1_sb, wf2_sb = [], []
                  for kc in range(KC):
                      w1 = wf1p.tile([128, DL], bf16, name=f"wf1_{kc}", tag=f"wf1_{kc}")
                      nc.sync.dma_start(w1[:], wfc1[kc * 128:(kc + 1) * 128, :])
                      wf1_sb.append(w1)
                      w2 = wf2p.tile([128, DL], bf16, name=f"wf2_{kc}", tag=f"wf2_{kc}")
                      nc.sync.dma_start(w2[:], wfc2[kc * 128:(kc + 1) * 128, :])
                      wf2_sb.append(w2)
                  wm_sb = []
                  off = 0
                  for oc, osz in enumerate(FCO):
                      w = wmpl.tile([osz, C], bf16, name=f"wm_{oc}", tag=f"wm_{oc}")
                      nc.sync.dma_start(w[:], wmp[off:off + osz, :])
                      wm_sb.append(w)
                      off += osz

                  for t4 in range(NT4):
                      x2 = []
                      pss = st_ps.tile([1, 512], f32, name="pss", tag="pss")
                      for cc in range(KC):
                          xt2 = x2p.tile([128, 512], bf16, name="x2t", tag="x2t")
                          nc.sync.dma_start(xt2[:],
                                            arout[t4][cc * 128:(cc + 1) * 128, :])
                          x2.append(xt2)
                          xq = xsqp.tile([128, 512], f32r, name="xq", tag="xq")
                          nc.vector.tensor_mul(xq[:], xt2[:], xt2[:])
                          nc.tensor.matmul(pss[:], ones_col[:], xq[:],
                                           start=(cc == 0), stop=(cc == KC - 1),
                                           skip_group_check=True)
                      sq = smalls.tile([1, 512], f32, name="sq", tag="sq")
                      nc.scalar.activation(sq[:], pss[:], FA.Sqrt,
                                           bias=eps_t[:], scale=1.0 / C)
                      rc2 = smalls.tile([1, 512], f32, name="rc2", tag="rc2")
                      nc.vector.reciprocal(rc2[:], sq[:])
                      rc2r = smalls.tile([1, 512], f32r, name="rc2r", tag="rc2r")
                      nc.scalar.copy(rc2r[:], rc2[:])
                      pb2 = st_ps.tile([128, 512], f32, name="pb2", tag="pb2")
                      nc.tensor.matmul(pb2[:], ones_row[:], rc2r[:],
                                       start=True, stop=True, skip_group_check=True)
                      rb2 = rb2p.tile([128, 512], f32, name="rb2", tag="rb2")
                      nc.scalar.copy(rb2[:], pb2[:])

                      xh2 = []
                      for cc in range(KC):
                          xh = xh2p.tile([128, 512], bf16, name="xh2", tag="xh2")
                          nc.vector.tensor_mul(xh[:], x2[cc][:], rb2[:])
                          xh2.append(xh)

                      g_tiles = []
                      off = 0
                      for oc, osz in enumerate(FCO):
                          pa = fc_ps.tile([128, 512], f32, name="pa", tag="fcps")
                          for kc in range(KC):
                              nc.tensor.matmul(pa[:osz], wf1_sb[kc][:, off:off + osz],
                                               xh2[kc][:], start=(kc == 0),
                                               stop=(kc == KC - 1))
                          a_sb = ap2.tile([128, 512], bf16, name="a_sb", tag="a_sb")
                          nc.scalar.activation(a_sb[:osz], pa[:osz], FA.Silu)
                          pb_ = fc_ps.tile([128, 512], f32, name="pb_", tag="fcps")
                          for kc in range(KC):
                              nc.tensor.matmul(pb_[:osz], wf2_sb[kc][:, off:off + osz],
                                               xh2[kc][:], start=(kc == 0),
                                               stop=(kc == KC - 1))
                          g_sb = gp.tile([128, 512], bf16, name="g_sb", tag="g_sb")
                          nc.vector.tensor_mul(g_sb[:osz], pb_[:osz], a_sb[:osz])
                          g_tiles.append(g_sb)
                          off += osz

                      for cc in range(KC):
                          pm = pm_ps.tile([128, 512], f32, name="pm", tag="pm")
                          for oc, osz in enumerate(FCO):
                              nc.tensor.matmul(
                                  pm[:], wm_sb[oc][:, cc * 128:(cc + 1) * 128],
                                  g_tiles[oc][:osz], start=(oc == 0),
                                  stop=(oc == len(FCO) - 1))
                          ob = ob2p.tile([128, 512], f32, name="ob2", tag="ob2")
                          nc.vector.scalar_tensor_tensor(
                              ob[:], x2[cc][:], 1.0 / NCORES, pm[:],
                              op0=ALU.mult, op1=ALU.add)
                          nc.sync.dma_start(rsin[t4][cc * 128:(cc + 1) * 128, :], ob[:])
                      nc.gpsimd.collective_compute(
                          "ReduceScatter", ALU.add, ins=[rsin[t4][:]],
                          outs=[rsout[t4][:]], replica_groups=RG)
                      nc.sync.dma_start(out[:, t4 * 512:(t4 + 1) * 512], rsout[t4][:])

    nc.compile()
    return nc


_NC = None


def _get_nc():
    global _NC
    if _NC is None:
        _NC = build()
    return _NC


def make_in_maps(x, alibi, w_attn, w_proj, w_fc1, w_fc2, w_mlp_proj,
                 rms1_scale, rms2_scale):
    x = np.asarray(x, dtype=np.float32)
    alibi = np.asarray(alibi, dtype=np.float32)
    w_attn = np.asarray(w_attn, dtype=np.float32)
    w_proj = np.asarray(w_proj, dtype=np.float32)
    w_fc1 = np.asarray(w_fc1, dtype=np.float32)
    w_fc2 = np.asarray(w_fc2, dtype=np.float32)
    w_mlp_proj = np.asarray(w_mlp_proj, dtype=np.float32)
    rms1_scale = np.asarray(rms1_scale, dtype=np.float32)
    rms2_scale = np.asarray(rms2_scale, dtype=np.float32)

    xt = np.ascontiguousarray(x[0].T)                       # [C, T]
    ms = np.mean(x[0].astype(np.float32) ** 2, axis=-1)     # [T]
    rs1 = (1.0 / np.sqrt(ms + EPS)).astype(np.float32)[None, :]

    # fold rms scales into the weight rows
    wA = w_attn * rms1_scale[:, None]
    wf1 = w_fc1 * rms2_scale[:, None]
    wf2 = w_fc2 * rms2_scale[:, None]

    # causal mask folded into the (transposed) alibi: [key, query] layout
    tk = np.arange(T)[:, None]
    q = np.arange(T)[None, :]
    cmaskT = np.where(tk <= q, 0.0, NEG).astype(np.float32)

    in_maps = []
    for r in range(NCORES):
        wqkv_r = np.ascontiguousarray(np.concatenate(
            [wA[:, r * CL:(r + 1) * CL],
             wA[:, C + r * CL:C + (r + 1) * CL],
             wA[:, 2 * C + r * CL:2 * C + (r + 1) * CL]], axis=1))
        in_maps.append({
            "xt": xt,
            "rs1": rs1,
            "wqkv": wqkv_r,
            "alibiT": (np.ascontiguousarray(
                alibi[r * HL:(r + 1) * HL].transpose(0, 2, 1))
                + cmaskT[None]).astype(ml_dtypes.bfloat16),
            "wproj": np.ascontiguousarray(w_proj[r * CL:(r + 1) * CL, :]),
            "wfc1": np.ascontiguousarray(
                wf1[:, r * DL:(r + 1) * DL]).astype(ml_dtypes.bfloat16),
            "wfc2": np.ascontiguousarray(
                wf2[:, r * DL:(r + 1) * DL]).astype(ml_dtypes.bfloat16),
            "wmp": np.ascontiguousarray(
                w_mlp_proj[r * DL:(r + 1) * DL, :]).astype(ml_dtypes.bfloat16),
        })
    return in_maps


def assemble(results):
    full = np.empty((T, C), dtype=np.float32)
    for r in range(NCORES):
        full[:, r * (C // NCORES):(r + 1) * (C // NCORES)] = results[r]["out"].T
    return full[None, :, :]


def kernel(x, alibi, w_attn, w_proj, w_fc1, w_fc2, w_mlp_proj,
           rms1_scale, rms2_scale):
    nc = _get_nc()
    in_maps = make_in_maps(x, alibi, w_attn, w_proj, w_fc1, w_fc2, w_mlp_proj,
                           rms1_scale, rms2_scale)
    res = run_bass_kernel_spmd(nc, in_maps, core_ids=list(range(NCORES)))
    return assemble(res.results)

